# revision 1
# baseline (speedup 1.0000x reference)
"""Trainium2 Bass kernel for the cross-batch retrieval contrastive loss.

Pipeline per batch b (reference semantics):
  sent_mean = mean(sent_feat * masks)                     (host)
  v1   = conv1([bef^T; broadcast sent_mean])              -> (196, 512)
  MHA over 196 positions, out_proj                        -> (196, 512)
  mod  = conv2(o)                                         -> (196, 512)
  ql   = mod @ q_w^T + q_b ; kl = aft @ k_w^T + k_b       -> (196, 512)
  logits[a,b,l,m] = ql[a,l,:] . kl[b,m,:]   (head split is a no-op)
  t2v[a,b] = mean_l max_m ; v2t[a,b] = mean_m max_l
  loss = symmetric InfoNCE on S = 0.5*(t2v+v2t)*exp(logit_scale)  (host, 32x32)

Sharding: data-parallel over the query-batch axis 'a' (4 batches/core x 8
cores). Each core computes kl for all 32 key batches (replicated), its own
front-end, and both orientations of every (a, b) logits tile so that the
max over l and the max over m are both free-axis DVE reductions.

Performance structure (the per-body cost is PE-sequencer bound at roughly
100 ns per PE instruction, so everything is shaped to minimize matmul
count):
- all projections run in fp8e4 with DoubleRow k-pairs (contraction 512 ->
  2 matmuls) on 392-wide (batch-pair) moving operands; activations stay
  in ranges where fp8 is safe via scales folded into weights on the host;
- attention (scores, softmax, attn@v) stays bf16; exp runs on ACT with
  the 1/8 score scale and a 1/32 colsum so the normalizer lands in fp8-
  friendly range for the o-tensor scale;
- biases are applied as per-partition bias operands of the PSUM->SBUF
  ACT copies (no rank-1 bias matmuls);
- logits: 784 query rows packed into 7 stationary tiles, 6272 key rows
  into 49; both orientations emitted as interleaved streams on separate
  PSUM pools; per-batch attribution of row maxes via 0/1 mask matmuls;
- final 32x32 InfoNCE on the host in float64 (tiny).
"""

import numpy as np
import ml_dtypes

B, LV, LT, D, H = 32, 196, 40, 512, 8
NCORES = 8
AL = B // NCORES          # query batches per core
KT = D // 128             # 128-row feature tiles per 512-dim tensor
LSPLIT = [(0, 128), (128, 68)]   # 196 = 128 + 68
NQ = AL * LV              # 784 query position-rows per core
NKEY = B * LV             # 6272 key position-rows
TQ = (NQ + 127) // 128    # 7 stationary tiles over query rows
TK = NKEY // 128          # 49 stationary tiles over key rows
W2 = 2 * LV               # batch-pair moving width
W2P = 400                 # fe fp8 tile stride (16B-aligned for DoubleRow)
BF16 = ml_dtypes.bfloat16
F8 = ml_dtypes.float8_e4m3fn

# fp8 scale plan (folded into weights/biases on the host; all activations
# then sit in fp8e4's comfortable range):
#   bef, v1, q, k, v: x1     o: x32 (via the 1/32 colsum)   pt: x32
#   ct: x32   ql: x128 (= FP8_SQ)   kl: x8 (= FP8_SK)
FP8_SQ = 128.0
FP8_SK = 8.0
S_OT = 32.0
S_PT = 32.0
S_CT = 32.0
# fp8 weight tensors: name -> host scale of the OUTPUT relative to its input
W8 = {
    "w1a8": 1.0,                # v1 = W1a . bef           (in x1)
    "wq8": 1.0,                 # q = Wq . v1
    "wk8": 1.0,
    "wv8": 1.0,
    "wo8": S_PT / S_OT,         # pt = Wo . ot
    "wc28": S_CT / S_PT,        # ct = Wc2 . pt
    "wql8": FP8_SQ / S_CT,      # ql = Wql . ct
    "wkl8": FP8_SK,             # kl = Wkl . aft
}

_CACHE = {}


def _build_program(reps=1):
    from contextlib import ExitStack
    import concourse.bacc as bacc
    import concourse.tile as tile
    from concourse import mybir

    f32 = mybir.dt.float32
    bf = mybir.dt.bfloat16
    f8 = mybir.dt.float8e4

    nc = bacc.Bacc("TRN2", target_bir_lowering=False, debug=False,
                   num_devices=NCORES)

    d = {
        "befT": nc.dram_tensor("befT", [128, KT, NQ], f8,
                               kind="ExternalInput").ap(),
        "aftT": nc.dram_tensor("aftT", [128, KT, NKEY], f8,
                               kind="ExternalInput").ap(),
        # txtc: per-partition conv1 text contribution, (128, KT, AL) f32
        "txtc": nc.dram_tensor("txtc", [128, KT * AL], f32,
                               kind="ExternalInput").ap(),
        # biases, partition-major (128, KT) f32
        "bc2": nc.dram_tensor("bc2", [128, KT], f32, kind="ExternalInput").ap(),
        "bql": nc.dram_tensor("bql", [128, KT], f32, kind="ExternalInput").ap(),
        "bkl": nc.dram_tensor("bkl", [128, KT], f32, kind="ExternalInput").ap(),
        "amask": nc.dram_tensor("amask", [128, TQ * AL], bf,
                                kind="ExternalInput").ap(),
        "bmask": nc.dram_tensor("bmask", [128, TK * B], bf,
                                kind="ExternalInput").ap(),
        "out": nc.dram_tensor("out", [2 * AL, B], f32, kind="ExternalOutput").ap(),
    }
    for n in W8:
        d[n] = nc.dram_tensor(n, [128, KT, D], f8, kind="ExternalInput").ap()

    with tile.TileContext(nc) as tc, ExitStack() as ctx:
        const = ctx.enter_context(tc.tile_pool(name="const", bufs=1))
        big = ctx.enter_context(tc.tile_pool(name="big", bufs=1))
        fe = ctx.enter_context(tc.tile_pool(name="fe", bufs=2))
        ps = ctx.enter_context(tc.tile_pool(name="ps", bufs=3, space="PSUM"))
        psb = ctx.enter_context(tc.tile_pool(name="psb", bufs=3, space="PSUM"))
        ps2 = ctx.enter_context(tc.tile_pool(name="ps2", bufs=2, space="PSUM"))

        for _rep in range(reps):
            _kernel_body(nc, tc, mybir, const, big, fe, ps, psb, ps2, d)

    nc.compile()
    return nc


def _kernel_body(nc, tc, mybir, const, big, fe, ps, psb, ps2, d):
    f32 = mybir.dt.float32
    bf = mybir.dt.bfloat16
    f8 = mybir.dt.float8e4
    AX = mybir.AxisListType.X
    MAX = mybir.AluOpType.max
    EXP = mybir.ActivationFunctionType.Exp
    IDENT = mybir.ActivationFunctionType.Identity
    DR = mybir.MatmulPerfMode.DoubleRow

    # ---- constants / weights into SBUF ----
    ones = const.tile([1, 64], bf, name="ones", tag="ones")
    nc.vector.memset(ones[:], 1.0)
    ones32 = const.tile([128, 1], bf, name="ones32", tag="ones32")
    nc.vector.memset(ones32[:], 1.0 / S_OT)

    txtc = const.tile([128, KT * AL], f32, name="txtc_sb", tag="txtc_sb")
    nc.sync.dma_start(out=txtc[:], in_=d["txtc"][:, :])
    amask = const.tile([128, TQ * AL], bf, name="amask_sb", tag="amask_sb")
    nc.sync.dma_start(out=amask[:], in_=d["amask"][:, :])
    bmask = const.tile([128, TK * B], bf, name="bmask_sb", tag="bmask_sb")
    nc.sync.dma_start(out=bmask[:], in_=d["bmask"][:, :])
    bias = {}
    for n in ["bc2", "bql", "bkl"]:
        bias[n] = const.tile([128, KT], f32, name=f"{n}_sb", tag=f"{n}_sb")
        nc.sync.dma_start(out=bias[n][:], in_=d[n][:, :])
    w = {}
    for n in W8:
        w[n] = const.tile([128, KT, D], f8, name=f"{n}_sb", tag=f"{n}_sb")
        nc.sync.dma_start(out=w[n][:], in_=d[n][:, :, :])

    aft = big.tile([128, KT, NKEY], f8, name="aft8", tag="aft8")
    nc.sync.dma_start(out=aft[:], in_=d["aftT"][:, :, :])
    befT = big.tile([128, KT, NQ], f8, name="bef8", tag="bef8")
    nc.sync.dma_start(out=befT[:], in_=d["befT"][:, :, :])
    klT = big.tile([128, KT, NKEY], f8, name="klT8", tag="klT8")
    qlT = big.tile([128, KT, NQ], f8, name="qlT8", tag="qlT8")

    def proj(dst, dst_col, src, src_col, wname, bname=None, txt_a=None,
             n=W2, pool=None):
        """dst[:, m, dst_col:+n] (fp8) = fp8-DR W^T x src[:, :, src_col:+n],
        bias (and conv1 text term) applied on the PSUM->SBUF ACT copy."""
        pool = pool or ps
        for m in range(KT):
            p = pool.tile([128, n], f32, name="p_proj", tag=pool.name)
            for j in range(KT // 2):
                nc.tensor.matmul(
                    p[:], lhsT=w[wname][:, 2 * j:2 * j + 2,
                                        m * 128:(m + 1) * 128],
                    rhs=src[:, 2 * j:2 * j + 2, src_col:src_col + n],
                    start=(j == 0), stop=(j == KT // 2 - 1), perf_mode=DR)
            out_ap = dst[:, m, dst_col:dst_col + n]
            if txt_a is not None:
                # conv1: per-batch text contribution as the copy's bias
                for ab in range(2):
                    a = txt_a + ab
                    nc.scalar.activation(
                        out_ap[:, ab * LV:(ab + 1) * LV],
                        p[:, ab * LV:(ab + 1) * LV], IDENT,
                        bias=txtc[:, a * KT + m: a * KT + m + 1])
            elif bname is not None:
                nc.scalar.activation(out_ap, p[:], IDENT,
                                     bias=bias[bname][:, m:m + 1])
            else:
                nc.scalar.copy(out_ap, p[:])

    # ---- phase 1: klT for all 32 key batches (fp8 DR) ----
    # output windows need no batch alignment; use full 512-wide moving ops
    for w0 in range(0, NKEY, 512):
        proj(klT, w0, aft, w0, "wkl8", bname="bkl", n=min(512, NKEY - w0))

    # ---- phase 2: front-end for the 4 local query batches (pairs) ----
    for apair in range(AL // 2):
        pc = apair * W2

        v1 = fe.tile([128, KT, W2P], f8, name="v1", tag="v1")
        proj(v1, 0, befT, pc, "w1a8", txt_a=apair * 2)
        qt = fe.tile([128, KT, W2P], f8, name="qt", tag="qt")
        kt = fe.tile([128, KT, W2P], f8, name="kt", tag="kt")
        proj(qt, 0, v1, 0, "wq8")
        proj(kt, 0, v1, 0, "wk8")

        ot = fe.tile([128, KT, W2P], f8, name="ot", tag="ot")
        for ab in range(2):
            ac = ab * LV
            # v position-major: (196, 512) as two row tiles (bf16)
            vpos = []
            for lt, (l0, ln) in enumerate(LSPLIT):
                p5 = ps.tile([ln, D], f32, name="p_vpos", tag="ps")
                for j in range(KT // 2):
                    nc.tensor.matmul(
                        p5[:], lhsT=v1[:, 2 * j:2 * j + 2, ac + l0:ac + l0 + ln],
                        rhs=w["wv8"][:, 2 * j:2 * j + 2, :],
                        start=(j == 0), stop=(j == KT // 2 - 1), perf_mode=DR)
                t = fe.tile([ln, D], bf, name=f"vpos_{lt}", tag=f"vpos_{lt}")
                nc.scalar.copy(t[:], p5[:])
                vpos.append(t)

            # attention, two heads per 128-partition group
            for kt2 in range(KT):
                po = ps.tile([128, LV], f32, name="p_o", tag="ps")
                pzb = ps.tile([128, LV], f32, name="p_zb", tag="ps")
                for hh in range(2):
                    h = kt2 * 2 + hh
                    off = 64 * hh
                    eT = []
                    for mt, (m0, mn) in enumerate(LSPLIT):
                        psc = psb.tile([mn, LV], f32, name="p_sc", tag="psb")
                        nc.tensor.matmul(
                            psc[:],
                            lhsT=kt[off:off + 64, kt2, ac + m0:ac + m0 + mn],
                            rhs=qt[off:off + 64, kt2, ac:ac + LV],
                            start=True, stop=True)
                        e = fe.tile([mn, LV], bf, name=f"eT_{mt}", tag=f"eT_{mt}")
                        nc.scalar.activation(e[:], psc[:], EXP, scale=0.125)
                        eT.append(e)
                    pz = ps2.tile([1, LV], f32, name="p_z", tag="ps2")
                    for mt, (m0, mn) in enumerate(LSPLIT):
                        nc.tensor.matmul(pz[:], lhsT=ones32[0:mn, 0:1],
                                         rhs=eT[mt][:], start=(mt == 0),
                                         stop=(mt == 1))
                    rz32 = fe.tile([1, LV], f32, name="rz32", tag="rz32")
                    nc.vector.reciprocal(rz32[:], pz[:])
                    rzb = fe.tile([1, LV], bf, name="rzb", tag="rzb")
                    nc.vector.tensor_copy(rzb[:], rz32[:])
                    nc.tensor.matmul(pzb[off:off + 64, :], lhsT=ones[0:1, 0:64],
                                     rhs=rzb[0:1, :], start=True, stop=True)
                    for mt, (m0, mn) in enumerate(LSPLIT):
                        nc.tensor.matmul(po[off:off + 64, :],
                                         lhsT=vpos[mt][:, h * 64:(h + 1) * 64],
                                         rhs=eT[mt][:], start=(mt == 0),
                                         stop=(mt == 1))
                zb = fe.tile([128, LV], f32, name="zb", tag="zb")
                nc.scalar.copy(zb[:], pzb[:])
                nc.vector.tensor_mul(ot[:, kt2, ac:ac + LV], po[:], zb[:])

        pt = fe.tile([128, KT, W2P], f8, name="pt", tag="pt")
        ct = fe.tile([128, KT, W2P], f8, name="ct", tag="ct")
        proj(pt, 0, ot, 0, "wo8")
        proj(ct, 0, pt, 0, "wc28", bname="bc2")
        proj(qlT, pc, ct, 0, "wql8", bname="bql")

    # ---- phase 3+4: both logits orientations, interleaved streams ----
    # orient 1: 784 query rows in 7 stationary tiles, rhs = key pairs.
    # orient 2: 6272 key rows in 49 stationary tiles, rhs = query pairs.
    # Separate PSUM pools so one stream's DVE reduces hide under the
    # other's matmuls.
    OSCALE = 1.0 / (LV * FP8_SQ * FP8_SK)
    t2v_sb = const.tile([AL, B], f32, name="t2v_sb", tag="t2v_sb")
    acc1 = fe.tile([AL, B], f32, name="acc1", tag="acc1")
    nc.vector.memset(acc1[:], 0.0)

    G = 3  # psum tiles per group (= pool bufs)

    def o1_stream():
        for t in range(TQ):
            q0 = t * 128
            qn = min(128, NQ - q0)
            rm = fe.tile([qn, B], bf, name="rm", tag="rm")
            for g0 in range(0, B // 2, G):
                bps = range(g0, min(g0 + G, B // 2))
                pts = [ps.tile([qn, W2], f32, name="p_lg", tag="ps")
                       for _ in bps]
                for j in range(KT // 2):
                    for i, bp in enumerate(bps):
                        nc.tensor.matmul(
                            pts[i][:], lhsT=qlT[:, 2 * j:2 * j + 2, q0:q0 + qn],
                            rhs=klT[:, 2 * j:2 * j + 2, bp * W2:(bp + 1) * W2],
                            start=(j == 0), stop=(j == KT // 2 - 1),
                            perf_mode=DR)
                for i, bp in enumerate(bps):
                    nc.vector.tensor_reduce(
                        rm[0:qn, bp * 2:bp * 2 + 2],
                        pts[i].rearrange("p (two n) -> p two n", two=2),
                        axis=AX, op=MAX)
                yield
            pacc = ps.tile([AL, B], f32, name="pacc", tag="ps")
            nc.tensor.matmul(pacc[:], lhsT=amask[0:qn, t * AL:(t + 1) * AL],
                             rhs=rm[:], start=True, stop=True)
            nc.vector.tensor_add(acc1[:], acc1[:], pacc[:])
        nc.scalar.mul(t2v_sb[:], acc1[:], OSCALE)
        nc.sync.dma_start(out=d["out"][0:AL, :], in_=t2v_sb[:])

    def o2_stream():
        # one shared colmax tile for all 4 local query batches, so the
        # per-batch attribution runs as a single 49-matmul chain at the end
        cm = fe.tile([128, TK, AL], bf, name="cm", tag="cm")
        for apair in range(AL // 2):
            pc = apair * W2
            for g0 in range(0, TK, G):
                tiles = range(g0, min(g0 + G, TK))
                pts = [psb.tile([128, W2], f32, name="p_lg2", tag="psb")
                       for _ in tiles]
                for j in range(KT // 2):
                    for i, t in enumerate(tiles):
                        nc.tensor.matmul(
                            pts[i][:], lhsT=klT[:, 2 * j:2 * j + 2,
                                                t * 128:(t + 1) * 128],
                            rhs=qlT[:, 2 * j:2 * j + 2, pc:pc + W2],
                            start=(j == 0), stop=(j == KT // 2 - 1),
                            perf_mode=DR)
                for i, t in enumerate(tiles):
                    nc.vector.tensor_reduce(
                        cm[:, t, 2 * apair:2 * apair + 2],
                        pts[i].rearrange("p (two n) -> p two n", two=2),
                        axis=AX, op=MAX)
                yield
        pv2 = ps2.tile([AL, B], f32, name="pv2", tag="ps2")
        for t in range(TK):
            nc.tensor.matmul(pv2[:], lhsT=cm[:, t, :],
                             rhs=bmask[:, t * B:(t + 1) * B],
                             start=(t == 0), stop=(t == TK - 1))
        v2t2 = fe.tile([AL, B], f32, name="v2t2", tag="v2t2")
        nc.scalar.mul(v2t2[:], pv2[:], OSCALE)
        nc.sync.dma_start(out=d["out"][AL:2 * AL, :], in_=v2t2[:])

    s1, s2 = o1_stream(), o2_stream()
    done1 = done2 = False
    while not (done1 and done2):
        if not done1:
            done1 = next(s1, "END") == "END"
        if not done2:
            done2 = next(s2, "END") == "END"


def get_program(reps=1):
    key = ("nc", reps)
    if key not in _CACHE:
        _CACHE[key] = _build_program(reps)
    return _CACHE[key]


def _to3d(mat512, cols, dtype):
    """(512, cols) feature-major -> (128, KT, cols) k-tile-major."""
    return np.ascontiguousarray(
        np.asarray(mat512, np.float32).reshape(KT, 128, cols)
        .transpose(1, 0, 2)).astype(dtype)


def make_in_maps(bef_feat, sent_feat, aft_feat, masks,
                 conv1_w, conv1_b, in_proj_w, out_proj_w, conv2_w, conv2_b,
                 q_w, q_b, k_w, k_b, logit_scale):
    bef_feat = np.asarray(bef_feat, np.float32)
    sent_feat = np.asarray(sent_feat, np.float32)
    aft_feat = np.asarray(aft_feat, np.float32)
    masks = np.asarray(masks, np.float32)
    conv1_w = np.asarray(conv1_w, np.float32)
    in_proj_w = np.asarray(in_proj_w, np.float32)

    sent_mean = (sent_feat * masks[:, :, None]).mean(axis=1)       # (B, D)
    txtc = sent_mean @ conv1_w[:, D:].T + np.asarray(conv1_b, np.float32)

    aftT = _to3d(aft_feat.transpose(2, 0, 1).reshape(D, NKEY), NKEY, F8)

    amask = np.zeros((128, TQ * AL), np.float32)
    for t in range(TQ):
        for r in range(min(128, NQ - t * 128)):
            amask[r, t * AL + (t * 128 + r) // LV] = 1.0
    bmask = np.zeros((128, TK * B), np.float32)
    for t in range(TK):
        for r in range(128):
            bmask[r, t * B + (t * 128 + r) // LV] = 1.0

    wmats = {
        "w1a8": _to3d(conv1_w[:, :D].T * W8["w1a8"], D, F8),
        "wq8": _to3d(in_proj_w[0:D, :].T * W8["wq8"], D, F8),
        "wk8": _to3d(in_proj_w[D:2 * D, :].T * W8["wk8"], D, F8),
        "wv8": _to3d(in_proj_w[2 * D:3 * D, :].T * W8["wv8"], D, F8),
        "wo8": _to3d(np.asarray(out_proj_w, np.float32).T * W8["wo8"], D, F8),
        "wc28": _to3d(np.asarray(conv2_w, np.float32).T * W8["wc28"], D, F8),
        "wql8": _to3d(np.asarray(q_w, np.float32).T * W8["wql8"], D, F8),
        "wkl8": _to3d(np.asarray(k_w, np.float32).T * W8["wkl8"], D, F8),
    }

    def pcol(vec, scale):
        # (D,) bias -> (128, KT) partition-major f32
        return np.ascontiguousarray(
            (np.asarray(vec, np.float32) * scale).reshape(KT, 128).T
        ).astype(np.float32)

    bvecs = {
        "bc2": pcol(conv2_b, S_CT),
        "bql": pcol(q_b, FP8_SQ),
        "bkl": pcol(k_b, FP8_SK),
    }

    in_maps = []
    for c in range(NCORES):
        sl = slice(c * AL, (c + 1) * AL)
        befT = _to3d(bef_feat[sl].transpose(2, 0, 1).reshape(D, NQ), NQ, F8)
        # txtc partition-major: (128, KT*AL), column a*KT + m
        tx = np.zeros((128, KT * AL), np.float32)
        for a in range(AL):
            tx[:, a * KT:(a + 1) * KT] = txtc[c * AL + a].reshape(KT, 128).T
        m = {"befT": befT, "aftT": aftT, "txtc": tx,
             "amask": amask.astype(BF16), "bmask": bmask.astype(BF16)}
        m.update(wmats)
        m.update(bvecs)
        in_maps.append(m)
    return in_maps


def finish(outs, logit_scale):
    """outs: list of 8 per-core (2*AL, B) arrays -> scalar loss."""
    t2v = np.zeros((B, B), np.float64)
    v2t = np.zeros((B, B), np.float64)
    for c in range(NCORES):
        o = np.asarray(outs[c], np.float64)
        t2v[c * AL:(c + 1) * AL, :] = o[0:AL]
        v2t[c * AL:(c + 1) * AL, :] = o[AL:2 * AL]
    S = 0.5 * (t2v + v2t) * np.exp(np.float64(np.asarray(logit_scale)))

    def ce(m):
        lse = np.log(np.sum(np.exp(m - m.max(axis=1, keepdims=True)), axis=1)) \
            + m.max(axis=1)
        return -np.mean(np.diag(m) - lse)

    loss = 0.5 * (ce(S) + ce(S.T))
    return np.float32(loss)


def kernel(**inputs):
    from concourse.bass_utils import run_bass_kernel_spmd

    nc = get_program()
    in_maps = make_in_maps(**inputs)
    res = run_bass_kernel_spmd(nc, in_maps, core_ids=list(range(NCORES)))
    outs = [res.results[c]["out"] for c in range(NCORES)]
    return finish(outs, inputs["logit_scale"])

